# revision 1
# baseline (speedup 1.0000x reference)
"""CDVAE encoder GNN — Trainium2 Bass kernel (8-core data-parallel over graphs).

Key structure (validated against the reference in fp64 to ~3e-6):
 - The coordinate-update pathway is dead code for (mu, logvar): coords only
   feed future coord updates; dist/ea are computed once from the original
   coords.  So the per-edge coord MLP is skipped entirely.
 - Edges are the full i!=j set within each 32-atom graph -> everything is
   computed dense per graph (32x32 pairs); the diagonal (i==i) columns of
   the stacked phase-B operand are ZEROED so h1_diag = 0 and silu(0) = 0
   contributes nothing to the j-sum (replaces per-tile poison memsets).
 - Edge-MLP layer 1 decomposes: h1[i,j] = a[i] + b[j] + c[i,j] with
   a = node@Wa + b1', b = node@Wb, c = sinfeat@Wc'.  edge_lin is folded
   into Wc' host-side: Wc' = elinw@Wc, b1' = b1 + elinb@Wc (exact algebra),
   so the device consumes raw sin/cos features.
 - a/b/sinfeat enter the edge matmul through one stacked K=128 operand
   (rows 0:64 sinfeat, 64:96 delta_i, 96:128 delta_j).
 - segment-sum commutes with the linear edge_w2: nm = (sum_j s1) @ W2 + 31*b2.
 - phase-B silu runs on [128, 2048] PSUM tiles (2 graphs) with bf16 output;
   the j-reduce runs on DVE at bf16 (2x) rate.

Sharding: 16 graphs per core, weights replicated, no collectives.
"""

import math
import numpy as np

import concourse.bass as bass
import concourse.mybir as mybir
import concourse.tile as tile
from concourse import bacc
from concourse.bass import ds, ts
from concourse.masks import make_identity

F32 = mybir.dt.float32
F32R = mybir.dt.float32r
BF16 = mybir.dt.bfloat16
I32 = mybir.dt.int32
AF = mybir.ActivationFunctionType
ALU = mybir.AluOpType

G_TOT = 128      # graphs total
NA = 32          # atoms per graph
GPC = 16         # graphs per core
NPC = GPC * NA   # nodes per core (512)
H = 512
ED = 64
L = 6
LAT2 = 512       # 2*latent
NCORES = 8

TWO_PI = 2.0 * math.pi
RNE_MAGIC = 1.5 * 2.0 ** 23          # fp32 round-to-nearest-int trick


def _r(ap):
    """bitcast an fp32 AP to float32r for full-rate matmul."""
    return ap.bitcast(F32R)


def build_module():
    """Build the per-core Bass module (same program on all 8 cores)."""
    nc = bacc.Bacc("TRN2", target_bir_lowering=False, debug=False)

    # ---- DRAM tensors (per-core inputs) ----
    dt_ = {}

    def din(name, shape, dtype=F32):
        t = nc.dram_tensor(name, list(shape), dtype, kind="ExternalInput")
        dt_[name] = t.ap()
        return dt_[name]

    coords_t = din("coords_t", (3, NPC))
    atypes = din("atypes", (NPC,), I32)
    aembed = din("aembed", (100, H))
    econst = din("econst", (ED, NA * NA))
    qs_pack = din("qs_pack", (4, ED))
    wa = din("wa", (L, H, H))
    wb = din("wb", (L, H, H))
    wcp = din("wcp", (L, ED, H))
    b1r = din("b1r", (L, 1, H))
    w2 = din("w2", (L, H, H))
    b2s = din("b2s", (L, 128, 4))
    wn1a = din("wn1a", (L, H, H))
    wn1b = din("wn1b", (L, H, H))
    nb1s = din("nb1s", (L, 128, 4))
    wn2 = din("wn2", (L, H, H))
    nb2s = din("nb2s", (L, 128, 4))
    gw1 = din("gw1", (H, H))
    gb1s = din("gb1s", (128, 4))
    gw2 = din("gw2", (H, LAT2))
    gb2s = din("gb2s", (128, 4))

    lat_out = nc.dram_tensor("lat", [LAT2, GPC], F32, kind="ExternalOutput").ap()
    if DEBUG_DUMPS:
        eac_dump = nc.dram_tensor(
            "eac_dump", [128, GPC * NA * NA], F32, kind="ExternalOutput").ap()
        wcab_dump = nc.dram_tensor(
            "wcab_dump", [128, GPC * H], F32, kind="ExternalOutput").ap()
        ssub_dump = nc.dram_tensor(
            "ssub_dump", [128, 4 * H], F32, kind="ExternalOutput").ap()
        node_dump = nc.dram_tensor(
            "node_dump", [128, 4 * H], F32, kind="ExternalOutput").ap()
        nrows_dump = nc.dram_tensor(
            "nrows_dump", [128, 4 * H], F32, kind="ExternalOutput").ap()
        at_dump = nc.dram_tensor(
            "at_dump", [128, 4], I32, kind="ExternalOutput").ap()

    with tile.TileContext(nc) as tc:
        with (
            tc.tile_pool(name="cpool", bufs=1) as cpool,
            tc.tile_pool(name="npool", bufs=2) as npool,
            tc.tile_pool(name="pa", bufs=2, space="PSUM") as pa,
        ):
            # ---------- persistent constants ----------
            # eac: rows 0:64 = sin-features (per graph), rows 64:96 = delta_i,
            # rows 96:128 = delta_j (replicated per graph) -> phase-B rhs is
            # ONE stacked K=128 operand.  Diagonal pair columns are zeroed.
            eac_sb = cpool.tile([128, GPC * NA * NA], F32, tag="eac")    # 64 KB/p
            qs_sb = cpool.tile([4, ED], F32, tag="qs")
            ct_sb = cpool.tile([3, NPC], F32, tag="ct")
            ones3 = cpool.tile([3, NA], F32, tag="ones3")
            ones1 = cpool.tile([1, 128], F32, tag="ones1")
            ident = cpool.tile([128, 128], F32, tag="ident")

            _mark(nc, "setup")
            # ---------- setup: node gather FIRST (before any large DMAs:
            # a big broadcast in flight breaks the gather's offset read) ----
            node0 = npool.tile([128, 4, H], F32, tag="node")
            with (
                tc.tile_pool(name="spool", bufs=1) as spool,
                tc.tile_pool(name="sp2", bufs=2) as sp2,
            ):
                nrows = spool.tile([128, 4 * H], F32, tag="nrows")
                at_sb = spool.tile([128, 4], I32, tag="at")
                for t in range(4):
                    nc.sync.dma_start(
                        out=at_sb[:, t : t + 1],
                        in_=atypes[ds(t * 128, 128)].rearrange("(p o) -> p o", o=1),
                    )
                for t in range(4):
                    nc.gpsimd.indirect_dma_start(
                        out=nrows[:, ds(t * H, H)],
                        out_offset=None,
                        in_=aembed,
                        in_offset=bass.IndirectOffsetOnAxis(
                            ap=at_sb[:, t : t + 1], axis=0
                        ),
                    )
                # delta_i / delta_j indicator rows (diag pre-zeroed host-side),
                # broadcast per graph from DRAM
                nc.sync.dma_start(
                    out=_r(eac_sb[ED:128, :].rearrange("p (g e) -> p g e", e=NA * NA)),
                    in_=_r(econst.unsqueeze(1).broadcast_to([ED, GPC, NA * NA])),
                )
                nc.sync.dma_start(out=_r(qs_sb[:]), in_=_r(qs_pack))
                nc.sync.dma_start(out=ct_sb[:], in_=coords_t)
                nc.vector.memset(ones3[:], 1.0)
                nc.vector.memset(ones1[:], 1.0)
                make_identity(nc, ident[:])
                if DEBUG_DUMPS:
                    nc.sync.dma_start(out=nrows_dump, in_=nrows[:])
                    nc.sync.dma_start(out=at_dump, in_=at_sb[:])
                # transpose 16 128x128 blocks: node0[hc*128+h, t*128+n] = node[n, h]
                for t in range(4):
                    pt = pa.tile([128, 512], F32, tag="pa")
                    for kc in range(4):
                        nc.tensor.transpose(
                            out=pt[:, ds(kc * 128, 128)],
                            in_=nrows[:, ds(t * H + kc * 128, 128)],
                            identity=ident[:],
                        )
                    nc.vector.tensor_copy(
                        out=_r(node0[:, :, ds(t * 128, 128)]),
                        in_=pt[:].rearrange("p (kc f) -> p kc f", f=128),
                    )

                # ---------- setup: distances ----------
                # Gram construction in pair-partition layout:
                # d2pp[gg*32+i, q*32+j] = |c_i - c_j|^2 of graph g = q*4+gg
                # (sq[i] + sq[j] - 2<ci,cj> via three accumulated matmuls)
                ctm2 = spool.tile([3, NPC], F32, tag="ctm2")
                ctsq = spool.tile([3, NPC], F32, tag="ctsq")
                nc.vector.tensor_scalar(
                    out=ctm2[:], in0=ct_sb[:], scalar1=-2.0, scalar2=None,
                    op0=ALU.mult,
                )
                nc.vector.tensor_tensor(
                    out=ctsq[:], in0=ct_sb[:], in1=ct_sb[:], op=ALU.mult
                )
                d2pp = spool.tile([128, 4 * NA], F32, tag="d2pp")
                for q in range(4):
                    pd = pa.tile([128, 512], F32, tag="pa")
                    for gg in range(4):
                        g = q * 4 + gg
                        csl = ds(g * NA, NA)
                        tp = (0, 32 * gg) if gg else None
                        osl = pd[ds(32 * gg, 32), 0:NA]
                        nc.tensor.matmul(
                            out=osl, lhsT=ct_sb[:, csl], rhs=ctm2[:, csl],
                            start=True, stop=False, tile_position=tp,
                        )
                        nc.tensor.matmul(
                            out=osl, lhsT=ctsq[:, csl], rhs=ones3[:],
                            start=False, stop=False, tile_position=tp,
                        )
                        nc.tensor.matmul(
                            out=osl, lhsT=ones3[:], rhs=ctsq[:, csl],
                            start=False, stop=True, tile_position=tp,
                        )
                    # clamp: diagonal d2 is ~0 (may round slightly negative)
                    nc.vector.tensor_scalar(
                        out=d2pp[:, ds(q * NA, NA)], in0=pd[:, 0:NA], scalar1=1e-12,
                        scalar2=None, op0=ALU.max,
                    )
                s0 = spool.tile([128, 4 * NA], F32, tag="s0")
                nc.scalar.activation(s0[:], d2pp[:], AF.Sqrt)
                rr = spool.tile([128, 4 * NA], F32, tag="rr")
                nc.vector.reciprocal(out=rr[:], in_=s0[:])
                t1 = spool.tile([128, 4 * NA], F32, tag="t1")
                nc.vector.tensor_tensor(
                    out=t1[:], in0=d2pp[:], in1=rr[:], op=ALU.mult
                )
                # dsum = s0 + d2/s0 = 2*dist (Newton); the x0.5 is folded in qs_pack
                dspp = spool.tile([128, 4 * NA], F32, tag="dspp")
                nc.vector.tensor_tensor(
                    out=dspp[:], in0=s0[:], in1=t1[:], op=ALU.add
                )
                # f32r matmuls round operands to ~12 mantissa bits, which
                # destroys the fractional part of tq = dist*f at |tq|~200.
                # Split dist (and f, host-side) into an 11-bit-exact high part
                # and a residual so every f32r product is (near-)exact.
                maskc = spool.tile([128, 1], I32, tag="maskc")
                nc.vector.memset(maskc[:], -4096)     # 0xFFFFF000
                dhpp = spool.tile([128, 4 * NA], F32, tag="dhpp")
                nc.vector.tensor_scalar(
                    out=dhpp[:].bitcast(I32), in0=dspp[:].bitcast(I32),
                    scalar1=maskc[:], scalar2=None,
                    op0=ALU.bitwise_and,
                )
                dlpp = spool.tile([128, 4 * NA], F32, tag="dlpp")
                nc.vector.tensor_tensor(
                    out=dlpp[:], in0=dspp[:], in1=dhpp[:], op=ALU.subtract
                )

                # ---------- setup: sin/cos features (batched per graph) ----
                # tq = dist*f (+0.25 on cos rows) via one K=2 matmul (dist row +
                # ones row); kk = RNE(tq); diff = tq - kk in [-.5,.5];
                # feat = Sin(2*pi*diff) written straight into eac rows 0:64.
                import os as _os
                _SKIP_SINF = _os.environ.get("SKIP_SINF", "0") == "1"
                _SKIP_DIAG = _os.environ.get("SKIP_DIAG", "0") == "1"
                for g in range(GPC if not _SKIP_SINF else 0):
                    psl = ds((g % 4) * NA, NA)
                    fsl = ds((g // 4) * NA, NA)
                    dsc = sp2.tile([4, NA * NA], F32, tag="dsc")
                    nc.vector.memset(dsc[:], 1.0)
                    nc.sync.dma_start(out=_r(dsc[0:1, :]), in_=_r(dhpp[psl, fsl]))
                    nc.sync.dma_start(out=_r(dsc[1:2, :]), in_=_r(dlpp[psl, fsl]))
                    nc.sync.dma_start(out=_r(dsc[2:3, :]), in_=_r(dhpp[psl, fsl]))
                    ptq = pa.tile([ED, NA * NA], F32, tag="pa")
                    for s in range(2):
                        nsl = ds(s * 512, 512)
                        nc.tensor.matmul(
                            out=ptq[:, nsl], lhsT=_r(qs_sb[:]),
                            rhs=_r(dsc[:, nsl]), start=True, stop=True,
                        )
                    kk = sp2.tile([ED, NA * NA], F32, tag="kk")
                    nc.vector.tensor_scalar(
                        out=kk[:], in0=ptq[:],
                        scalar1=RNE_MAGIC, scalar2=RNE_MAGIC,
                        op0=ALU.add, op1=ALU.subtract,
                    )
                    df = sp2.tile([ED, NA * NA], F32, tag="df")
                    nc.vector.scalar_tensor_tensor(
                        out=df[:], in0=kk[:], scalar=-1.0,
                        in1=ptq[:], op0=ALU.mult, op1=ALU.add,
                    )
                    nc.scalar.activation(
                        _r(eac_sb[0:ED, ds(g * NA * NA, NA * NA)]),
                        df[:], AF.Sin, scale=TWO_PI,
                    )
                # zero the diagonal (i==j) sin-feature columns: together with
                # the pre-zeroed indicator diag, h1_diag = 0 and silu(0) = 0
                for g in range(GPC if not _SKIP_DIAG else 0):
                    nc.vector.memset(
                        eac_sb[0:ED, g * NA * NA : (g + 1) * NA * NA : NA + 1], 0.0
                    )

            # ---------- layer-phase pools (opened after setup scratch frees) ----------
            from contextlib import ExitStack as _ES

            lctx = _ES()
            wpool = lctx.enter_context(tc.tile_pool(name="wpool", bufs=3))
            wsm = lctx.enter_context(tc.tile_pool(name="wsm", bufs=2))
            work = lctx.enter_context(tc.tile_pool(name="work", bufs=3))

            # ---------- persistent layer-loop tensors ----------
            # wcab: phase-B stationary [Wc'(64); a_g(32); b_g(32)] per graph
            wcab = cpool.tile([128, GPC, H], F32, tag="wcab")
            ssub = cpool.tile([128, 4, H], F32, tag="ssub")
            nm_sb = cpool.tile([128, 4, H], F32, tag="nm")
            z1s = cpool.tile([128, 4, H], F32, tag="z1s")

            if DEBUG_DUMPS:
                nc.sync.dma_start(out=eac_dump, in_=eac_sb[:])
                nc.sync.dma_start(
                    out=node_dump.rearrange("p (c m) -> p c m", c=4), in_=node0[:])

            node = node0
            for l in range(L):
                _mark(nc, f"L{l}.A")
                # ---- phase A: a/b projections (row layout, per graph) ----
                wa_t = wpool.tile([128, 4, H], F32, tag="wbig")
                nc.sync.dma_start(
                    out=_r(wa_t[:]), in_=_r(wa[l].rearrange("(c p) m -> p c m", p=128))
                )
                wb_t = wpool.tile([128, 4, H], F32, tag="wbig")
                nc.sync.dma_start(
                    out=_r(wb_t[:]), in_=_r(wb[l].rearrange("(c p) m -> p c m", p=128))
                )
                b1r_t = wsm.tile([1, H], F32, tag="b1r")
                nc.sync.dma_start(out=_r(b1r_t[:]), in_=_r(b1r[l]))

                # Wc' -> wcab rows 0:64 (replicated per graph)
                nc.sync.dma_start(
                    out=_r(wcab[0:ED, :, :]),
                    in_=_r(wcp[l].unsqueeze(1).broadcast_to([ED, GPC, H])),
                )
                for t in range(4):  # 4 graphs per batch (M=128)
                    pab = pa.tile([128, 512], F32, tag="pa")
                    for kc in range(4):
                        nc.tensor.matmul(
                            out=pab[:],
                            lhsT=_r(node[:, kc, ds(t * 128, 128)]),
                            rhs=_r(wa_t[:, kc, :]),
                            start=(kc == 0), stop=False,
                        )
                    nc.tensor.matmul(
                        out=pab[:], lhsT=_r(ones1[:]), rhs=_r(b1r_t[:]),
                        start=False, stop=True,
                    )
                    pbt = pa.tile([128, 512], F32, tag="pa")
                    for kc in range(4):
                        nc.tensor.matmul(
                            out=pbt[:],
                            lhsT=_r(node[:, kc, ds(t * 128, 128)]),
                            rhs=_r(wb_t[:, kc, :]),
                            start=(kc == 0), stop=(kc == 3),
                        )
                    sta = work.tile([128, 512], F32, tag="stg")
                    nc.vector.tensor_copy(out=sta[:], in_=pab[:])
                    stb = work.tile([128, 512], F32, tag="stg")
                    nc.vector.tensor_copy(out=stb[:], in_=pbt[:])
                    for gg in range(4):
                        g = t * 4 + gg
                        nc.sync.dma_start(
                            out=_r(wcab[ED : ED + NA, g, :]),
                            in_=_r(sta[ds(gg * NA, NA), :]),
                        )
                        nc.sync.dma_start(
                            out=_r(wcab[ED + NA : 128, g, :]),
                            in_=_r(stb[ds(gg * NA, NA), :]),
                        )

                if DEBUG_DUMPS and l == 0:
                    nc.sync.dma_start(
                        out=wcab_dump.rearrange("p (g m) -> p g m", g=GPC),
                        in_=wcab[:])

                _mark(nc, f"L{l}.B")
                # ---- phase B: dense edge pass, 2 graphs per PSUM tile ----
                for hc in range(4):
                    hsl = ds(hc * 128, 128)
                    for gp in range(0, GPC, 2):
                        ph = pa.tile([128, 2048], F32, tag="pa")
                        for gi in range(2):
                            g = gp + gi
                            for s in range(2):
                                nc.tensor.matmul(
                                    out=ph[:, ds(gi * 1024 + s * 512, 512)],
                                    lhsT=_r(wcab[:, g, hsl]),
                                    rhs=_r(eac_sb[:, ds(g * 1024 + s * 512, 512)]),
                                    start=True, stop=True,
                                )
                        sg = work.tile([128, 2048], BF16, tag="sg")
                        nc.scalar.activation(sg[:], ph[:], AF.Silu)
                        with nc.allow_low_precision(reason="f32r round on write"):
                            nc.vector.tensor_reduce(
                                out=_r(ssub[:, hc, ds(gp * NA, 2 * NA)]),
                                in_=sg[:].rearrange("p (i j) -> p i j", j=NA),
                                op=ALU.add, axis=mybir.AxisListType.X,
                            )

                if DEBUG_DUMPS and l == 0:
                    nc.sync.dma_start(
                        out=ssub_dump.rearrange("p (c m) -> p c m", c=4),
                        in_=ssub[:])

                _mark(nc, f"L{l}.C")
                # ---- phase C: nm + node MLP ----
                w2_t = wpool.tile([128, 4, H], F32, tag="wbig")
                nc.sync.dma_start(
                    out=_r(w2_t[:]), in_=_r(w2[l].rearrange("(c p) m -> p c m", p=128))
                )
                b2s_t = wsm.tile([128, 4], F32, tag="bias")
                nc.sync.dma_start(out=b2s_t[:], in_=b2s[l])
                for hc in range(4):
                    pn = pa.tile([128, 512], F32, tag="pa")
                    for kc in range(4):
                        nc.tensor.matmul(
                            out=pn[:],
                            lhsT=_r(w2_t[:, kc, ds(hc * 128, 128)]),
                            rhs=_r(ssub[:, kc, :]),
                            start=(kc == 0), stop=(kc == 3),
                        )
                    nc.scalar.activation(
                        _r(nm_sb[:, hc, :]), pn[:], AF.Identity,
                        bias=b2s_t[:, hc : hc + 1], scale=1.0,
                    )

                wn1a_t = wpool.tile([128, 4, H], F32, tag="wbig")
                nc.sync.dma_start(
                    out=_r(wn1a_t[:]), in_=_r(wn1a[l].rearrange("(c p) m -> p c m", p=128))
                )
                wn1b_t = wpool.tile([128, 4, H], F32, tag="wbig")
                nc.sync.dma_start(
                    out=_r(wn1b_t[:]), in_=_r(wn1b[l].rearrange("(c p) m -> p c m", p=128))
                )
                nb1s_t = wsm.tile([128, 4], F32, tag="bias")
                nc.sync.dma_start(out=nb1s_t[:], in_=nb1s[l])
                for hc in range(4):
                    pz = pa.tile([128, 512], F32, tag="pa")
                    for kc in range(4):
                        nc.tensor.matmul(
                            out=pz[:],
                            lhsT=_r(wn1a_t[:, kc, ds(hc * 128, 128)]),
                            rhs=_r(node[:, kc, :]),
                            start=(kc == 0), stop=False,
                        )
                    for kc in range(4):
                        nc.tensor.matmul(
                            out=pz[:],
                            lhsT=_r(wn1b_t[:, kc, ds(hc * 128, 128)]),
                            rhs=_r(nm_sb[:, kc, :]),
                            start=False, stop=(kc == 3),
                        )
                    nc.scalar.activation(
                        _r(z1s[:, hc, :]), pz[:], AF.Silu,
                        bias=nb1s_t[:, hc : hc + 1], scale=1.0,
                    )

                wn2_t = wpool.tile([128, 4, H], F32, tag="wbig")
                nc.sync.dma_start(
                    out=_r(wn2_t[:]), in_=_r(wn2[l].rearrange("(c p) m -> p c m", p=128))
                )
                nb2s_t = wsm.tile([128, 4], F32, tag="bias")
                nc.sync.dma_start(out=nb2s_t[:], in_=nb2s[l])
                node_next = npool.tile([128, 4, H], F32, tag="node")
                for hc in range(4):
                    pz2 = pa.tile([128, 512], F32, tag="pa")
                    for kc in range(4):
                        nc.tensor.matmul(
                            out=pz2[:],
                            lhsT=_r(wn2_t[:, kc, ds(hc * 128, 128)]),
                            rhs=_r(z1s[:, kc, :]),
                            start=(kc == 0), stop=(kc == 3),
                        )
                    nc.scalar.activation(
                        _r(node_next[:, hc, :]), pz2[:], AF.Identity,
                        bias=nb2s_t[:, hc : hc + 1], scale=1.0,
                    )
                node = node_next

            _mark(nc, "final")
            # ---------- final: graph pool + latent MLP ----------
            graph_t = cpool.tile([128, 4, GPC], F32, tag="graph")
            for hc in range(4):
                with nc.allow_low_precision(reason="f32r round on write"):
                    nc.vector.tensor_reduce(
                        out=_r(graph_t[:, hc, :]),
                        in_=node[:, hc, :].rearrange("p (g a) -> p g a", a=NA),
                        op=ALU.add, axis=mybir.AxisListType.X,
                    )
            gw1_t = wpool.tile([128, 4, H], F32, tag="wbig")
            nc.sync.dma_start(
                out=_r(gw1_t[:]), in_=_r(gw1.rearrange("(c p) m -> p c m", p=128))
            )
            gb1s_t = wsm.tile([128, 4], F32, tag="bias")
            nc.sync.dma_start(out=gb1s_t[:], in_=gb1s)
            q1s = cpool.tile([128, 4, GPC], F32, tag="q1s")
            for hc in range(4):
                pq = pa.tile([128, 512], F32, tag="pa")
                for kc in range(4):
                    nc.tensor.matmul(
                        out=pq[:, 0:GPC],
                        lhsT=_r(gw1_t[:, kc, ds(hc * 128, 128)]),
                        rhs=_r(graph_t[:, kc, :]),
                        start=(kc == 0), stop=(kc == 3),
                    )
                nc.scalar.activation(
                    _r(q1s[:, hc, :]), pq[:, 0:GPC], AF.Silu,
                    bias=gb1s_t[:, hc : hc + 1], scale=1.0,
                )

            gw2_t = wpool.tile([128, 4, LAT2], F32, tag="wbig")
            nc.sync.dma_start(
                out=_r(gw2_t[:]), in_=_r(gw2.rearrange("(c p) m -> p c m", p=128))
            )
            gb2s_t = wsm.tile([128, 4], F32, tag="bias")
            nc.sync.dma_start(out=gb2s_t[:], in_=gb2s)
            lat_sb = cpool.tile([128, 4, GPC], F32, tag="lat")
            for oc in range(4):
                pl = pa.tile([128, 512], F32, tag="pa")
                for kc in range(4):
                    nc.tensor.matmul(
                        out=pl[:, 0:GPC],
                        lhsT=_r(gw2_t[:, kc, ds(oc * 128, 128)]),
                        rhs=_r(q1s[:, kc, :]),
                        start=(kc == 0), stop=(kc == 3),
                    )
                nc.scalar.activation(
                    lat_sb[:, oc, :], pl[:, 0:GPC], AF.Identity,
                    bias=gb2s_t[:, oc : oc + 1], scale=1.0,
                )
            nc.sync.dma_start(
                out=lat_out.rearrange("(c p) g -> p c g", p=128), in_=lat_sb[:]
            )
            lctx.close()

    nc.compile()
    return nc


def prep_inputs(inputs):
    """Host-side packing: shard per core + weight layout transforms."""
    f32 = np.float32
    coords = np.asarray(inputs["coords"], f32)
    atom_types = np.asarray(inputs["atom_types"], np.int32)
    ew1 = np.asarray(inputs["edge_w1"], f32)
    eb1 = np.asarray(inputs["edge_b1"], f32)
    elinw = np.asarray(inputs["edge_lin_w"], f32)
    elinb = np.asarray(inputs["edge_lin_b"], f32)
    wc_raw = np.ascontiguousarray(ew1[:, 2 * H :, :])          # (L, 64, 512)

    def chunk_bias(b):  # (L?,512) -> (...,128,4) per-partition chunks
        b = np.asarray(b, f32)
        if b.ndim == 1:
            return np.ascontiguousarray(b.reshape(4, 128).T)
        return np.ascontiguousarray(b.reshape(-1, 4, 128).transpose(0, 2, 1))

    # fold edge_lin into Wc: c = feat @ (elinw @ Wc) + elinb @ Wc
    wcp = np.ascontiguousarray(np.einsum("fe,leh->lfh", elinw, wc_raw))
    b1p = eb1 + np.einsum("e,leh->lh", elinb, wc_raw)          # (L, 512)

    # indicator matrix: rows 0-31 delta(i), rows 32-63 delta(j); the
    # diagonal (i==j) columns are zeroed so h1_diag = 0 -> silu(0) = 0
    ec = np.zeros((ED, NA * NA), f32)
    ii, jj = np.meshgrid(np.arange(NA), np.arange(NA), indexing="ij")
    ii, jj = ii.ravel(), jj.ravel()
    m = ii != jj
    ec[ii[m], np.arange(NA * NA)[m]] = 1.0
    ec[32 + jj[m], np.arange(NA * NA)[m]] = 1.0

    gfp = np.asarray(inputs["gfp_W"], f32)
    # tq = dist*f (+0.25 turn on cos rows), computed as an exact f32r
    # matmul via hi/lo mantissa splits: rhs rows are (d_hi, d_lo, d_hi,
    # ones), so lhsT rows must be (f_hi, f_hi, f_lo, qshift).
    fq = np.concatenate([gfp, gfp]) * np.float32(0.5)
    f_hi = (fq.view(np.uint32) & np.uint32(0xFFFFF000)).view(np.float32)
    f_lo = (fq - f_hi).astype(f32)
    qs_pack = np.stack([
        f_hi, f_hi, f_lo,
        np.concatenate([np.zeros(32, f32), np.full(32, 0.25, f32)]),
    ]).astype(f32)

    shared = {
        "aembed": np.ascontiguousarray(np.asarray(inputs["atom_embed"], f32)),
        "econst": ec,
        "qs_pack": np.ascontiguousarray(qs_pack),
        "wa": np.ascontiguousarray(ew1[:, :H, :]),
        "wb": np.ascontiguousarray(ew1[:, H : 2 * H, :]),
        "wcp": wcp,
        "b1r": np.ascontiguousarray(b1p.reshape(L, 1, H)),
        "w2": np.ascontiguousarray(np.asarray(inputs["edge_w2"], f32)),
        "b2s": chunk_bias(np.asarray(inputs["edge_b2"], f32) * (NA - 1)),
        "wn1a": np.ascontiguousarray(np.asarray(inputs["node_w1"], f32)[:, :H, :]),
        "wn1b": np.ascontiguousarray(np.asarray(inputs["node_w1"], f32)[:, H:, :]),
        "nb1s": chunk_bias(inputs["node_b1"]),
        "wn2": np.ascontiguousarray(np.asarray(inputs["node_w2"], f32)),
        "nb2s": chunk_bias(inputs["node_b2"]),
        "gw1": np.ascontiguousarray(np.asarray(inputs["graph_w1"], f32) / NA),
        "gb1s": chunk_bias(inputs["graph_b1"]),
        "gw2": np.ascontiguousarray(np.asarray(inputs["graph_w2"], f32)),
        "gb2s": chunk_bias(inputs["graph_b2"]),
    }

    in_maps = []
    for c in range(NCORES):
        sl = slice(c * NPC, (c + 1) * NPC)
        m = dict(shared)
        m["coords_t"] = np.ascontiguousarray(coords[sl].T)
        m["atypes"] = np.ascontiguousarray(atom_types[sl])
        in_maps.append(m)
    return in_maps


_CACHE = {}
PHASE_MARKS = []
DEBUG_DUMPS = False


def _mark(nc, name):
    PHASE_MARKS.append((name, nc.next_id()))


def kernel(**inputs):
    from concourse import bass_utils

    if "nc" not in _CACHE:
        _CACHE["nc"] = build_module()
    nc = _CACHE["nc"]
    in_maps = prep_inputs(inputs)
    res = bass_utils.run_bass_kernel_spmd(
        nc, in_maps, core_ids=list(range(NCORES))
    )
    lat = np.concatenate(
        [res.results[c]["lat"].T for c in range(NCORES)], axis=0
    )  # (128, 512)
    mu, logvar = lat[:, : LAT2 // 2], lat[:, LAT2 // 2 :]
    return (mu, logvar)

